# revision 21
# baseline (speedup 1.0000x reference)
"""Multi-head attention (B=4, S=2048, D=1024, H=16) on 8 TRN2 NeuronCores.

Sharding: core c -> (batch b = c//2, head-group g = c%2 of 8 heads).
Data parallel over batch, tensor parallel over heads; each core computes
its group's QKV projection slices, causal attention for its 8 heads, and
the partial output projection. Host sums the two per-batch partials
(the tensor-parallel unshard) and adds the V-bias epilogue.

Fused causal pipeline: the QKV projection of sequence block b overlaps
the attention of q-tile b-1 (causality: q-tile qi only needs K/V blocks
<= qi), so the tensor engine never idles while the scalar engine chews
softmax exps and the HAM clock gate stays at 8/8. The output projection
of q-tile qi is emitted interleaved into q-tile qi+1's head loop to
fill the ACT-bound stretches; dummy warmup matmuls during the DMA
prologue pre-warm the PE clock. Softmax normalization uses a DVE
reciprocal + GPSIMD partition_broadcast (no PE broadcast matmul, no
PSUM traffic, no DVE copies). On-device layout is "features on
partitions" throughout; scores are computed in transposed form
S.T[k, q] so exp'd probabilities feed the PV matmul directly, with the
softmax denominator riding the PV matmul as a leading ones-column of V.
Causal masking uses suffix-restricted tiles plus small multiplicative
0/1 strips on the exp'd probabilities; non-staircase masks fall back to
additive -1e9 biases on the scores. Output partials are stored f16 and
summed f32 on the host.

PSUM budget (8 banks): projection ring 2 + score ring 2x2 + PV/O-proj
ring 2. Engine balance per core (slowest): PE ~265us busy (floor ~228:
proj 82 + scores 60 + PV 60 + O-proj 27 -- scores/PV pass counts are
output-column-bound, verified on HW that tile_position row-packing does
NOT stream concurrently), ACT ~163us (145k exp lane-cycles + 160 call
overheads; call count locked by the 2-bank score-tile geometry), DVE
~120us, GPSIMD ~33us. fp8 was measured (numpy, e4m3 on any matmul
operand) at 1.8-4.7e-2 max-normalized error -- over the 2e-2 gate, so
all matmuls stay f16/f32-accumulate.
"""

import os
import numpy as np

B, S, D, H = 4, 2048, 1024, 16
DK = D // H          # 64
HPC = H // 2         # heads per core = 8
GD = HPC * DK        # group feature width = 512
QT = 512             # q-tile width (free dim of S.T chunks)
KTL = 128            # k-tile length (partition dim of S.T chunks)
N_QT = S // QT       # 4
N_KT = S // KTL      # 16
SB = 512             # projection seq block (= QT)
STRW = 128           # padded width of multiplicative mask strips
NEG = np.float32(-1e9)
SCALE = 1.0 / np.sqrt(np.float32(DK))

_cache = {}
last_results = None


def _classify_mask(mask2d):
    """Classify each (q-tile, k-tile) block of the [S,S] bool mask.

    Returns (plan, strips, biases):
      plan[qi] = list over valid kt of (kt, kind, a, b):
        kind 0 = clean (no masking)
        kind 1 = staircase: a = q0 (suffix start), b = (strip_idx, strip_w)
        kind 2 = general:   a = bias_idx
      strips: list of [KTL, STRW] f32 0/1 multiplicative masks (padded)
      biases: list of [KTL, QT] f32 additive -1e9/0 masks
    Blocks are in S.T (k, q) layout.
    """
    kl = np.arange(KTL)[:, None]
    ql = np.arange(QT)[None, :]
    plan = []
    strips, strip_keys = [], {}
    biases, bias_keys = [], {}
    for qi in range(N_QT):
        row = []
        for kt in range(N_KT):
            blk = mask2d[qi * QT:(qi + 1) * QT, kt * KTL:(kt + 1) * KTL].T
            if blk.all():
                continue
            if not blk.any():
                row.append((kt, 0, 0, None))
                continue
            dj = kt * KTL - qi * QT
            stair = (0 <= dj <= QT - KTL and np.array_equal(blk, kl + dj > ql)
                     and not os.environ.get("KERNEL_NO_STAIR"))
            q0 = min(dj, QT - STRW) if stair else 0
            if stair and (q0 == 0 or row):
                w = min(dj + KTL, QT) - q0
                pat = (~blk[:, q0:q0 + w]).astype(np.float32)
                key = (w, pat.tobytes())
                if key not in strip_keys:
                    strip_keys[key] = len(strips)
                    p = np.zeros((KTL, STRW), np.float32)
                    p[:, :w] = pat
                    strips.append(p)
                row.append((kt, 1, q0, (strip_keys[key], w)))
            else:
                bias = np.where(blk, NEG, np.float32(0.0))
                key = bias.tobytes()
                if key not in bias_keys:
                    bias_keys[key] = len(biases)
                    biases.append(bias)
                row.append((kt, 2, bias_keys[key], None))
        if not row:
            # fully-masked q-row: include everything with full bias so the
            # softmax matches the reference's uniform distribution.
            bias = np.full((KTL, QT), NEG, np.float32)
            key = bias.tobytes()
            if key not in bias_keys:
                bias_keys[key] = len(biases)
                biases.append(bias)
            row = [(kt, 2, bias_keys[key], None) for kt in range(N_KT)]
        plan.append(row)
    return plan, strips, biases


def _build(plan, n_strips, n_biases):
    import concourse.bass as bass
    import concourse.bacc as bacc
    import concourse.tile as tile
    import concourse.mybir as mybir
    from contextlib import ExitStack

    f32 = mybir.dt.float32
    f16 = mybir.dt.float16
    Exp = mybir.ActivationFunctionType.Exp

    nc = bacc.Bacc(trn_type="TRN2", target_bir_lowering=False, debug=False)
    xT = nc.dram_tensor("xT", [D, S], f16, kind="ExternalInput").ap()
    w_qk = nc.dram_tensor("w_qk", [D, 2 * GD], f16, kind="ExternalInput").ap()
    b_qk = nc.dram_tensor("b_qk", [2 * GD], f32, kind="ExternalInput").ap()
    w_v = nc.dram_tensor("w_v", [D, GD], f16, kind="ExternalInput").ap()
    wo_T = nc.dram_tensor("wo_T", [GD, D], f16, kind="ExternalInput").ap()
    maskm = nc.dram_tensor("maskm", [max(n_strips, 1), KTL, STRW], f16,
                           kind="ExternalInput").ap()
    maskb = nc.dram_tensor("maskb", [max(n_biases, 1), KTL, QT], f32,
                           kind="ExternalInput").ap()
    outT = nc.dram_tensor("outT", [D, S], f16, kind="ExternalOutput").ap()

    ND = D // 128       # 8 contraction chunks
    NM = 2 * GD // 128  # 8 QK feature chunks (0-3 = Q.T, 4-7 = K.T)
    NK3 = GD // 128     # 4 output-projection contraction chunks

    with tile.TileContext(nc) as tc, ExitStack() as ctx:
        singles = ctx.enter_context(tc.tile_pool(name="singles", bufs=1))
        qkt_pool = ctx.enter_context(tc.tile_pool(name="qkt", bufs=1))
        x_pool = ctx.enter_context(tc.tile_pool(name="xp", bufs=1))
        v_pool = ctx.enter_context(tc.tile_pool(name="vp", bufs=1))
        pt_pool = ctx.enter_context(tc.tile_pool(name="pt", bufs=8))
        rr_pool = ctx.enter_context(tc.tile_pool(name="rr", bufs=4))
        otq_pool = ctx.enter_context(tc.tile_pool(name="otq", bufs=2))
        p3o = ctx.enter_context(tc.tile_pool(name="p3o", bufs=4))
        # PSUM: proj ring 2 banks + scores ring 4 banks + ot/oproj ring 2
        pP = ctx.enter_context(tc.tile_pool(name="pP", bufs=2, space="PSUM"))
        st_pool = ctx.enter_context(tc.tile_pool(name="st", bufs=2, space="PSUM"))
        ot_pool = ctx.enter_context(tc.tile_pool(name="ot", bufs=2, space="PSUM"))

        qkt = [qkt_pool.tile([128, S], f16, tag=f"qkt{m}", name=f"qkt{m}")
               for m in range(NM)]
        xs = [x_pool.tile([128, S], f16, tag=f"x{k}", name=f"xs{k}")
              for k in range(ND)]
        v_sb = [v_pool.tile([128, HPC, 128], f16, tag=f"v{t}", name=f"v{t}")
                for t in range(N_KT)]
        bqk_t = singles.tile([128, NM], f32)
        wqk_t = [singles.tile([128, 2 * GD], f16, tag=f"wqk{k}", name=f"wqk{k}")
                 for k in range(ND)]
        wv_t = [singles.tile([128, GD], f16, tag=f"wv{k}", name=f"wv{k}")
                for k in range(ND)]
        wo_t = [singles.tile([128, D], f16, tag=f"wo{k}", name=f"wo{k}")
                for k in range(NK3)]

        # ---- prologue DMAs, ordered so proj(0) can start earliest; spread
        # across engine DMA queues so descriptor processing parallelizes ----
        for k in range(ND):
            weng = nc.scalar if k % 2 == 0 else nc.gpsimd
            weng.dma_start(out=wqk_t[k], in_=w_qk[128 * k:128 * (k + 1)])
            nc.sync.dma_start(out=xs[k][:, 0:SB],
                              in_=xT[128 * k:128 * (k + 1), 0:SB])
        nc.sync.dma_start(out=bqk_t, in_=b_qk.rearrange("(m p) -> p m", p=128))
        for k in range(ND):
            nc.scalar.dma_start(out=wv_t[k], in_=w_v[128 * k:128 * (k + 1)])
        for b in range(1, N_QT):
            for k in range(ND):
                nc.sync.dma_start(out=xs[k][:, SB * b:SB * (b + 1)],
                                  in_=xT[128 * k:128 * (k + 1),
                                         SB * b:SB * (b + 1)])
        for k in range(NK3):
            nc.scalar.dma_start(out=wo_t[k], in_=wo_T[128 * k:128 * (k + 1)])
        # PE warmup: ~3.5us of dummy matmuls during the DMA wait flips the
        # HAM clock gate to 8/8 before the first real projection matmul.
        wu = singles.tile([128, SB], f16, tag="wu", name="wu")
        nc.vector.memset(wu, 0.0)
        for i in range(20):
            ps = pP.tile([128, SB], f32, tag="pP", name=f"wu{i}")
            nc.tensor.matmul(ps[:], wu[:, 0:128], wu[:], start=True, stop=True)
        mm_t = []
        for i in range(n_strips):
            t = singles.tile([KTL, STRW], f16, tag=f"mm{i}", name=f"mm{i}")
            nc.sync.dma_start(out=t, in_=maskm[i])
            mm_t.append(t)
        mb_t = []
        for i in range(n_biases):
            t = singles.tile([KTL, QT], f32, tag=f"mb{i}", name=f"mb{i}")
            nc.sync.dma_start(out=t, in_=maskb[i])
            mb_t.append(t)

        # otq[qi % 2][hp]: normalized attention output, features on partitions
        def otq_tiles():
            return [otq_pool.tile([128, QT], f16, tag=f"otq{m}", name=f"otq{m}")
                    for m in range(NK3)]

        def emit_proj_qk(b, m):
            ps = pP.tile([128, SB], f32, tag="pP", name=f"ps_qk{b}_{m}")
            for k in range(ND):
                nc.tensor.matmul(
                    ps[:], wqk_t[k][:, 128 * m:128 * (m + 1)],
                    xs[k][:, SB * b:SB * (b + 1)],
                    start=(k == 0), stop=(k == ND - 1))
            nc.vector.tensor_scalar_add(
                qkt[m][:, SB * b:SB * (b + 1)], ps[:], bqk_t[:, m:m + 1])

        def emit_proj_v(b):
            for tt in range(SB // 128):
                t = b * (SB // 128) + tt
                ps = pP.tile([128, GD], f32, tag="pP", name=f"ps_v{t}")
                for k in range(ND):
                    nc.tensor.matmul(
                        ps[:], xs[k][:, SB * b + 128 * tt:SB * b + 128 * (tt + 1)],
                        wv_t[k][:], start=(k == 0), stop=(k == ND - 1))
                nc.vector.tensor_copy(
                    out=v_sb[t][:, :, DK:2 * DK],
                    in_=ps[:].rearrange("p (h d) -> p h d", h=HPC))
                nc.vector.memset(v_sb[t][:, :, 0:1], 1.0)
                nc.vector.memset(v_sb[t][:, :, 1:DK], 0.0)

        def emit_proj_part(b, j):
            """Q/K feature chunks for head-pair j of block b (+ V on j=0).

            Emitted inside attention hp-slots so projection distributes as
            tensor-engine filler across the whole preceding attention span;
            attention of q-tile b's hp j only needs chunks {j, 4+j}."""
            emit_proj_qk(b, j)
            emit_proj_qk(b, 4 + j)
            if j == 0:
                emit_proj_v(b)

        def emit_proj(b):
            for j in range(4):
                emit_proj_part(b, j)

        def emit_oproj_chunks(otq, qi, ms):
            """Output-projection chunks m in ms for q-tile qi."""
            for m in ms:
                ps = pP.tile([128, QT], f32, tag="pP", name=f"ps_o{qi}_{m}")
                for k in range(NK3):
                    nc.tensor.matmul(
                        ps[:], wo_t[k][:, 128 * m:128 * (m + 1)], otq[k][:],
                        start=(k == 0), stop=(k == NK3 - 1))
                ob = p3o.tile([128, QT], f16, tag="ob", name="ob")
                nc.vector.tensor_copy(out=ob[:], in_=ps[:])
                nc.sync.dma_start(
                    out=outT[128 * m:128 * (m + 1), QT * qi:QT * (qi + 1)],
                    in_=ob[:])

        def emit_attn(qi, deferred):
            """Attention for q-tile qi; `deferred` = (otq, prev_qi) whose
            output projection is interleaved 2 chunks per head-pair.

            This q-tile's own projection parts j>=1 are emitted inside the
            hp j-1 slot (just-in-time, so they fill THIS q-tile's ACT-paced
            gaps instead of being front-run by the greedy scheduler), and
            the next block's hp0 needs (m0/m4/V) go in the hp3 slot."""
            kts = plan[qi]
            otq = otq_tiles()
            for hp in range(HPC // 2):
                ot_ps = [ot_pool.tile([128, QT], f32, tag="ot", name="ot_ps")
                         for _ in range(2)]
                for ki, (kt, kind, a, bopt) in enumerate(kts):
                    q0 = a if kind == 1 else 0
                    st = st_pool.tile([128, 2, QT], f32, tag="st", name="st")
                    for h in range(2):
                        lo, hi = 64 * h, 64 * h + 64
                        nc.tensor.matmul(
                            st[:, h, q0:QT],
                            qkt[4 + hp][lo:hi, KTL * kt:KTL * (kt + 1)],
                            qkt[hp][lo:hi, QT * qi + q0:QT * (qi + 1)],
                            start=True, stop=True, tile_position=(64 * h, 0))
                    if kind == 2:
                        for h in range(2):
                            nc.vector.tensor_add(st[:, h, :], st[:, h, :],
                                                 mb_t[a][:])
                    pt = pt_pool.tile([128, 2, QT], f16, tag="pt", name="pt")
                    nc.scalar.activation(out=pt[:, :, q0:QT], in_=st[:, :, q0:QT],
                                         func=Exp, scale=float(SCALE))
                    if kind == 1:
                        si, w = bopt
                        for h in range(2):
                            nc.vector.tensor_mul(pt[:, h, q0:q0 + w],
                                                 pt[:, h, q0:q0 + w],
                                                 mm_t[si][:, 0:w])
                    for h in range(2):
                        nc.tensor.matmul(
                            ot_ps[h][:, q0:QT],
                            v_sb[kt][:, 2 * hp + h, :],
                            pt[:, h, q0:QT],
                            start=(ki == 0), stop=(ki == len(kts) - 1))
                for h in range(2):
                    r_row = rr_pool.tile([1, QT], f32, tag="rrow", name="r_row")
                    nc.vector.reciprocal_approx_fast(out=r_row[:],
                                                     in_=ot_ps[h][0:1, :])
                    rb_sb = rr_pool.tile([DK, QT], f32, tag="rbsb", name="rb_sb")
                    nc.gpsimd.partition_broadcast(rb_sb[:], r_row[:])
                    nc.vector.tensor_mul(otq[hp][64 * h:64 * h + 64, :],
                                         ot_ps[h][DK:2 * DK, :], rb_sb[:])
                if deferred is not None:
                    emit_oproj_chunks(deferred[0], deferred[1],
                                      range(2 * hp, 2 * hp + 2))
                if hp < 3:
                    emit_proj_part(qi, hp + 1)
                elif qi + 1 < N_QT:
                    emit_proj_part(qi + 1, 0)
            return otq

        emit_proj_part(0, 0)
        prev = None
        for qi in range(N_QT):
            otq = emit_attn(qi, prev)
            prev = (otq, qi)
        emit_oproj_chunks(prev[0], prev[1], range(D // 128))
    nc.compile()
    return nc


def kernel(encodings_for_qkv, mask, w_qkv, b_qkv, w_o):
    global last_results
    from concourse.bass_utils import run_bass_kernel_spmd

    x = np.ascontiguousarray(np.asarray(encodings_for_qkv, dtype=np.float32))
    mask2d = np.asarray(mask).reshape(S, S).astype(bool)
    w_qkv = np.asarray(w_qkv, dtype=np.float32)
    b_qkv = np.asarray(b_qkv, dtype=np.float32)
    w_o = np.asarray(w_o, dtype=np.float32)

    plan, strips, biases = _classify_mask(mask2d)
    key = repr([[e[:3] + ((e[3][0], e[3][1]) if e[3] else None,) for e in row]
                for row in plan]) + repr(sorted(
                    (k, v) for k, v in os.environ.items() if k.startswith("KERNEL_")))
    if key not in _cache:
        _cache[key] = _build(plan, len(strips), len(biases))
    nc = _cache[key]

    maskm = (np.stack(strips) if strips
             else np.zeros((1, KTL, STRW), dtype=np.float32))
    maskb = (np.stack(biases) if biases
             else np.zeros((1, KTL, QT), dtype=np.float32))
    wT = np.ascontiguousarray(w_qkv.T)        # [D, 3D]
    woT_full = w_o.T                          # [D(in), D(out)]

    in_maps = []
    for c in range(8):
        b, g = divmod(c, 2)
        cols = slice(GD * g, GD * (g + 1))
        w_qk_g = np.ascontiguousarray(
            np.concatenate([wT[:, 0 * D:][:, cols], wT[:, 1 * D:][:, cols]], axis=1))
        b_qk_g = np.ascontiguousarray(
            np.concatenate([b_qkv[0 * D:1 * D][cols], b_qkv[1 * D:2 * D][cols]]))
        w_v_g = np.ascontiguousarray(wT[:, 2 * D:][:, cols])
        wo_T_g = np.ascontiguousarray(woT_full[cols, :])
        in_maps.append({
            "xT": np.ascontiguousarray(x[b].T).astype(np.float16),
            "w_qk": w_qk_g.astype(np.float16), "b_qk": b_qk_g,
            "w_v": w_v_g.astype(np.float16),
            "wo_T": wo_T_g.astype(np.float16),
            "maskm": maskm.astype(np.float16), "maskb": maskb,
        })

    trace = bool(int(os.environ.get("KERNEL_PROFILE", "0")))
    res = run_bass_kernel_spmd(nc, in_maps, core_ids=list(range(8)),
                               trace=trace,
                               trace_cores=list(range(8)) if trace else None)
    last_results = res

    out = np.empty((B, S, D), dtype=np.float32)
    for b in range(B):
        acc = (res.results[2 * b]["outT"].astype(np.float32)
               + res.results[2 * b + 1]["outT"].astype(np.float32))
        out[b] = acc.T
    # V-bias epilogue: softmax rows sum to 1, so the V bias contributes a
    # constant (b_v @ w_o.T) to every sequence position.
    out += (b_qkv[2 * D:] @ woT_full).reshape(1, 1, D)
    return out


# revision 23
# speedup vs baseline: 1.0411x; 1.0411x over previous
"""Multi-head attention (B=4, S=2048, D=1024, H=16) on 8 TRN2 NeuronCores.

Sharding: core c -> (batch b = c//2, head-group g = c%2 of 8 heads).
Data parallel over batch, tensor parallel over heads; each core computes
its group's QKV projection slices, causal attention for its 8 heads, and
the partial output projection. Host sums the two per-batch partials
(the tensor-parallel unshard) and adds the V-bias epilogue.

Fused causal pipeline: the QKV projection of sequence block b overlaps
the attention of q-tile b-1 (causality: q-tile qi only needs K/V blocks
<= qi), so the tensor engine never idles while the scalar engine chews
softmax exps and the HAM clock gate stays at 8/8. The output projection
of q-tile qi is emitted interleaved into q-tile qi+1's head loop to
fill the ACT-bound stretches; dummy warmup matmuls during the DMA
prologue pre-warm the PE clock. Softmax normalization uses a DVE
reciprocal + GPSIMD partition_broadcast (no PE broadcast matmul, no
PSUM traffic, no DVE copies). On-device layout is "features on
partitions" throughout; scores are computed in transposed form
S.T[k, q] so exp'd probabilities feed the PV matmul directly, with the
softmax denominator riding the PV matmul as a leading ones-column of V.
Causal masking uses suffix-restricted tiles plus small multiplicative
0/1 strips on the exp'd probabilities; non-staircase masks fall back to
additive -1e9 biases on the scores. Output partials are stored f16 and
summed f32 on the host.

PSUM budget (8 banks): projection ring 2 + score ring 2x2 + PV/O-proj
ring 2. Engine balance per core (slowest): PE ~265us busy (floor ~228:
proj 82 + scores 60 + PV 60 + O-proj 27 -- scores/PV pass counts are
output-column-bound, verified on HW that tile_position row-packing does
NOT stream concurrently), ACT ~163us (145k exp lane-cycles + 160 call
overheads; call count locked by the 2-bank score-tile geometry), DVE
~120us, GPSIMD ~33us. fp8 was measured (numpy, e4m3 on any matmul
operand) at 1.8-4.7e-2 max-normalized error -- over the 2e-2 gate, so
all matmuls stay f16/f32-accumulate.
"""

import os
import numpy as np

B, S, D, H = 4, 2048, 1024, 16
DK = D // H          # 64
HPC = H // 2         # heads per core = 8
GD = HPC * DK        # group feature width = 512
QT = 512             # q-tile width (free dim of S.T chunks)
KTL = 128            # k-tile length (partition dim of S.T chunks)
N_QT = S // QT       # 4
N_KT = S // KTL      # 16
SB = 512             # projection seq block (= QT)
STRW = 128           # padded width of multiplicative mask strips
NEG = np.float32(-1e9)
SCALE = 1.0 / np.sqrt(np.float32(DK))

_cache = {}
last_results = None


def _classify_mask(mask2d):
    """Classify each (q-tile, k-tile) block of the [S,S] bool mask.

    Returns (plan, strips, biases):
      plan[qi] = list over valid kt of (kt, kind, a, b):
        kind 0 = clean (no masking)
        kind 1 = staircase: a = q0 (suffix start), b = (strip_idx, strip_w)
        kind 2 = general:   a = bias_idx
      strips: list of [KTL, STRW] f32 0/1 multiplicative masks (padded)
      biases: list of [KTL, QT] f32 additive -1e9/0 masks
    Blocks are in S.T (k, q) layout.
    """
    kl = np.arange(KTL)[:, None]
    ql = np.arange(QT)[None, :]
    plan = []
    strips, strip_keys = [], {}
    biases, bias_keys = [], {}
    for qi in range(N_QT):
        row = []
        for kt in range(N_KT):
            blk = mask2d[qi * QT:(qi + 1) * QT, kt * KTL:(kt + 1) * KTL].T
            if blk.all():
                continue
            if not blk.any():
                row.append((kt, 0, 0, None))
                continue
            dj = kt * KTL - qi * QT
            stair = (0 <= dj <= QT - KTL and np.array_equal(blk, kl + dj > ql)
                     and not os.environ.get("KERNEL_NO_STAIR"))
            q0 = min(dj, QT - STRW) if stair else 0
            if stair and (q0 == 0 or row):
                w = min(dj + KTL, QT) - q0
                pat = (~blk[:, q0:q0 + w]).astype(np.float32)
                key = (w, pat.tobytes())
                if key not in strip_keys:
                    strip_keys[key] = len(strips)
                    p = np.zeros((KTL, STRW), np.float32)
                    p[:, :w] = pat
                    strips.append(p)
                row.append((kt, 1, q0, (strip_keys[key], w)))
            else:
                bias = np.where(blk, NEG, np.float32(0.0))
                key = bias.tobytes()
                if key not in bias_keys:
                    bias_keys[key] = len(biases)
                    biases.append(bias)
                row.append((kt, 2, bias_keys[key], None))
        if not row:
            # fully-masked q-row: include everything with full bias so the
            # softmax matches the reference's uniform distribution.
            bias = np.full((KTL, QT), NEG, np.float32)
            key = bias.tobytes()
            if key not in bias_keys:
                bias_keys[key] = len(biases)
                biases.append(bias)
            row = [(kt, 2, bias_keys[key], None) for kt in range(N_KT)]
        plan.append(row)
    return plan, strips, biases


def _build(plan, n_strips, n_biases):
    import concourse.bass as bass
    import concourse.bacc as bacc
    import concourse.tile as tile
    import concourse.mybir as mybir
    from contextlib import ExitStack

    f32 = mybir.dt.float32
    f16 = mybir.dt.float16
    Exp = mybir.ActivationFunctionType.Exp

    nc = bacc.Bacc(trn_type="TRN2", target_bir_lowering=False, debug=False)
    xT = nc.dram_tensor("xT", [D, S], f16, kind="ExternalInput").ap()
    w_qk = nc.dram_tensor("w_qk", [D, 2 * GD], f16, kind="ExternalInput").ap()
    b_qk = nc.dram_tensor("b_qk", [2 * GD], f32, kind="ExternalInput").ap()
    w_v = nc.dram_tensor("w_v", [D, GD], f16, kind="ExternalInput").ap()
    wo_T = nc.dram_tensor("wo_T", [GD, D], f16, kind="ExternalInput").ap()
    maskm = nc.dram_tensor("maskm", [max(n_strips, 1), KTL, STRW], f16,
                           kind="ExternalInput").ap()
    maskb = nc.dram_tensor("maskb", [max(n_biases, 1), KTL, QT], f32,
                           kind="ExternalInput").ap()
    outT = nc.dram_tensor("outT", [D, S], f16, kind="ExternalOutput").ap()

    ND = D // 128       # 8 contraction chunks
    NM = 2 * GD // 128  # 8 QK feature chunks (0-3 = Q.T, 4-7 = K.T)
    NK3 = GD // 128     # 4 output-projection contraction chunks

    with tile.TileContext(nc) as tc, ExitStack() as ctx:
        singles = ctx.enter_context(tc.tile_pool(name="singles", bufs=1))
        qkt_pool = ctx.enter_context(tc.tile_pool(name="qkt", bufs=1))
        x_pool = ctx.enter_context(tc.tile_pool(name="xp", bufs=1))
        v_pool = ctx.enter_context(tc.tile_pool(name="vp", bufs=1))
        pt_pool = ctx.enter_context(tc.tile_pool(name="pt", bufs=6))
        rr_pool = ctx.enter_context(tc.tile_pool(name="rr", bufs=4))
        otq_pool = ctx.enter_context(tc.tile_pool(name="otq", bufs=2))
        p3o = ctx.enter_context(tc.tile_pool(name="p3o", bufs=4))
        # PSUM: proj ring 2 banks + scores ring 4 banks + ot/oproj ring 2
        pP = ctx.enter_context(tc.tile_pool(name="pP", bufs=2, space="PSUM"))
        st_pool = ctx.enter_context(tc.tile_pool(name="st", bufs=2, space="PSUM"))
        ot_pool = ctx.enter_context(tc.tile_pool(name="ot", bufs=2, space="PSUM"))

        qkt = [qkt_pool.tile([128, S], f16, tag=f"qkt{m}", name=f"qkt{m}")
               for m in range(NM)]
        xs = [x_pool.tile([128, S], f16, tag=f"x{k}", name=f"xs{k}")
              for k in range(ND)]
        v_sb = [v_pool.tile([128, HPC, 128], f16, tag=f"v{t}", name=f"v{t}")
                for t in range(N_KT)]
        bqk_t = singles.tile([128, NM], f32)
        wqk_t = [singles.tile([128, 2 * GD], f16, tag=f"wqk{k}", name=f"wqk{k}")
                 for k in range(ND)]
        wv_t = [singles.tile([128, GD], f16, tag=f"wv{k}", name=f"wv{k}")
                for k in range(ND)]
        wo_t = [singles.tile([128, D], f16, tag=f"wo{k}", name=f"wo{k}")
                for k in range(NK3)]

        # ---- prologue DMAs, ordered so proj(0) can start earliest; spread
        # across engine DMA queues so descriptor processing parallelizes ----
        for k in range(ND):
            weng = nc.scalar if k % 2 == 0 else nc.gpsimd
            weng.dma_start(out=wqk_t[k], in_=w_qk[128 * k:128 * (k + 1)])
            nc.sync.dma_start(out=xs[k][:, 0:SB],
                              in_=xT[128 * k:128 * (k + 1), 0:SB])
        nc.sync.dma_start(out=bqk_t, in_=b_qk.rearrange("(m p) -> p m", p=128))
        for k in range(ND):
            nc.scalar.dma_start(out=wv_t[k], in_=w_v[128 * k:128 * (k + 1)])
        for b in range(1, N_QT):
            for k in range(ND):
                nc.sync.dma_start(out=xs[k][:, SB * b:SB * (b + 1)],
                                  in_=xT[128 * k:128 * (k + 1),
                                         SB * b:SB * (b + 1)])
        for k in range(NK3):
            nc.scalar.dma_start(out=wo_t[k], in_=wo_T[128 * k:128 * (k + 1)])
        # PE warmup: ~3.5us of dummy matmuls during the DMA wait flips the
        # HAM clock gate to 8/8 before the first real projection matmul.
        wu = singles.tile([128, SB], f16, tag="wu", name="wu")
        nc.vector.memset(wu, 0.0)
        for i in range(20):
            ps = pP.tile([128, SB], f32, tag="pP", name=f"wu{i}")
            nc.tensor.matmul(ps[:], wu[:, 0:128], wu[:], start=True, stop=True)
        mm_t = []
        for i in range(n_strips):
            t = singles.tile([KTL, STRW], f16, tag=f"mm{i}", name=f"mm{i}")
            nc.sync.dma_start(out=t, in_=maskm[i])
            mm_t.append(t)
        mb_t = []
        for i in range(n_biases):
            t = singles.tile([KTL, QT], f32, tag=f"mb{i}", name=f"mb{i}")
            nc.sync.dma_start(out=t, in_=maskb[i])
            mb_t.append(t)

        # otq[qi % 2][hp]: normalized attention output, features on partitions
        def otq_tiles():
            return [otq_pool.tile([128, QT], f16, tag=f"otq{m}", name=f"otq{m}")
                    for m in range(NK3)]

        def emit_proj_qk(b, m):
            ps = pP.tile([128, SB], f32, tag="pP", name=f"ps_qk{b}_{m}")
            for k in range(ND):
                nc.tensor.matmul(
                    ps[:], wqk_t[k][:, 128 * m:128 * (m + 1)],
                    xs[k][:, SB * b:SB * (b + 1)],
                    start=(k == 0), stop=(k == ND - 1))
            nc.vector.tensor_scalar_add(
                qkt[m][:, SB * b:SB * (b + 1)], ps[:], bqk_t[:, m:m + 1])

        def emit_proj_v(b):
            for tt in range(SB // 128):
                t = b * (SB // 128) + tt
                ps = pP.tile([128, GD], f32, tag="pP", name=f"ps_v{t}")
                for k in range(ND):
                    nc.tensor.matmul(
                        ps[:], xs[k][:, SB * b + 128 * tt:SB * b + 128 * (tt + 1)],
                        wv_t[k][:], start=(k == 0), stop=(k == ND - 1))
                nc.vector.tensor_copy(
                    out=v_sb[t][:, :, DK:2 * DK],
                    in_=ps[:].rearrange("p (h d) -> p h d", h=HPC))
                nc.vector.memset(v_sb[t][:, :, 0:1], 1.0)
                nc.vector.memset(v_sb[t][:, :, 1:DK], 0.0)

        def emit_proj_part(b, j):
            """Q/K feature chunks for head-pair j of block b (+ V on j=0).

            Emitted inside attention hp-slots so projection distributes as
            tensor-engine filler across the whole preceding attention span;
            attention of q-tile b's hp j only needs chunks {j, 4+j}."""
            emit_proj_qk(b, j)
            emit_proj_qk(b, 4 + j)
            if j == 0:
                emit_proj_v(b)

        def emit_proj(b):
            for j in range(4):
                emit_proj_part(b, j)

        def emit_oproj_chunks(otq, qi, ms):
            """Output-projection chunks m in ms for q-tile qi."""
            for m in ms:
                ps = pP.tile([128, QT], f32, tag="pP", name=f"ps_o{qi}_{m}")
                for k in range(NK3):
                    nc.tensor.matmul(
                        ps[:], wo_t[k][:, 128 * m:128 * (m + 1)], otq[k][:],
                        start=(k == 0), stop=(k == NK3 - 1))
                ob = p3o.tile([128, QT], f16, tag="ob", name="ob")
                nc.vector.tensor_copy(out=ob[:], in_=ps[:])
                nc.sync.dma_start(
                    out=outT[128 * m:128 * (m + 1), QT * qi:QT * (qi + 1)],
                    in_=ob[:])

        def emit_attn(qi, deferred):
            """Attention for q-tile qi; `deferred` = (otq, prev_qi) whose
            output projection is interleaved 2 chunks per head-pair."""
            kts = plan[qi]
            otq = otq_tiles()
            for hp in range(HPC // 2):
                ot_ps = [ot_pool.tile([128, QT], f32, tag="ot", name="ot_ps")
                         for _ in range(2)]
                for ki, (kt, kind, a, bopt) in enumerate(kts):
                    q0 = a if kind == 1 else 0
                    st = st_pool.tile([128, 2, QT], f32, tag="st", name="st")
                    for h in range(2):
                        lo, hi = 64 * h, 64 * h + 64
                        nc.tensor.matmul(
                            st[:, h, q0:QT],
                            qkt[4 + hp][lo:hi, KTL * kt:KTL * (kt + 1)],
                            qkt[hp][lo:hi, QT * qi + q0:QT * (qi + 1)],
                            start=True, stop=True, tile_position=(64 * h, 0))
                    if kind == 2:
                        for h in range(2):
                            nc.vector.tensor_add(st[:, h, :], st[:, h, :],
                                                 mb_t[a][:])
                    pt = pt_pool.tile([128, 2, QT], f16, tag="pt", name="pt")
                    nc.scalar.activation(out=pt[:, :, q0:QT], in_=st[:, :, q0:QT],
                                         func=Exp, scale=float(SCALE))
                    if kind == 1:
                        si, w = bopt
                        for h in range(2):
                            nc.vector.tensor_mul(pt[:, h, q0:q0 + w],
                                                 pt[:, h, q0:q0 + w],
                                                 mm_t[si][:, 0:w])
                    for h in range(2):
                        nc.tensor.matmul(
                            ot_ps[h][:, q0:QT],
                            v_sb[kt][:, 2 * hp + h, :],
                            pt[:, h, q0:QT],
                            start=(ki == 0), stop=(ki == len(kts) - 1))
                for h in range(2):
                    r_row = rr_pool.tile([1, QT], f32, tag="rrow", name="r_row")
                    nc.vector.reciprocal_approx_fast(out=r_row[:],
                                                     in_=ot_ps[h][0:1, :])
                    rb_sb = rr_pool.tile([DK, QT], f32, tag="rbsb", name="rb_sb")
                    nc.gpsimd.partition_broadcast(rb_sb[:], r_row[:])
                    nc.vector.tensor_mul(otq[hp][64 * h:64 * h + 64, :],
                                         ot_ps[h][DK:2 * DK, :], rb_sb[:])
                if deferred is not None:
                    emit_oproj_chunks(deferred[0], deferred[1],
                                      range(2 * hp, 2 * hp + 2))
            return otq

        emit_proj(0)
        prev = None
        for qi in range(N_QT):
            otq = emit_attn(qi, prev)
            prev = (otq, qi)
            if qi + 1 < N_QT:
                emit_proj(qi + 1)
        emit_oproj_chunks(prev[0], prev[1], range(D // 128))
    nc.compile()
    return nc


def kernel(encodings_for_qkv, mask, w_qkv, b_qkv, w_o):
    global last_results
    from concourse.bass_utils import run_bass_kernel_spmd

    x = np.ascontiguousarray(np.asarray(encodings_for_qkv, dtype=np.float32))
    mask2d = np.asarray(mask).reshape(S, S).astype(bool)
    w_qkv = np.asarray(w_qkv, dtype=np.float32)
    b_qkv = np.asarray(b_qkv, dtype=np.float32)
    w_o = np.asarray(w_o, dtype=np.float32)

    plan, strips, biases = _classify_mask(mask2d)
    key = repr([[e[:3] + ((e[3][0], e[3][1]) if e[3] else None,) for e in row]
                for row in plan]) + repr(sorted(
                    (k, v) for k, v in os.environ.items() if k.startswith("KERNEL_")))
    if key not in _cache:
        _cache[key] = _build(plan, len(strips), len(biases))
    nc = _cache[key]

    maskm = (np.stack(strips) if strips
             else np.zeros((1, KTL, STRW), dtype=np.float32))
    maskb = (np.stack(biases) if biases
             else np.zeros((1, KTL, QT), dtype=np.float32))
    wT = np.ascontiguousarray(w_qkv.T)        # [D, 3D]
    woT_full = w_o.T                          # [D(in), D(out)]

    in_maps = []
    for c in range(8):
        b, g = divmod(c, 2)
        cols = slice(GD * g, GD * (g + 1))
        w_qk_g = np.ascontiguousarray(
            np.concatenate([wT[:, 0 * D:][:, cols], wT[:, 1 * D:][:, cols]], axis=1))
        b_qk_g = np.ascontiguousarray(
            np.concatenate([b_qkv[0 * D:1 * D][cols], b_qkv[1 * D:2 * D][cols]]))
        w_v_g = np.ascontiguousarray(wT[:, 2 * D:][:, cols])
        wo_T_g = np.ascontiguousarray(woT_full[cols, :])
        in_maps.append({
            "xT": np.ascontiguousarray(x[b].T).astype(np.float16),
            "w_qk": w_qk_g.astype(np.float16), "b_qk": b_qk_g,
            "w_v": w_v_g.astype(np.float16),
            "wo_T": wo_T_g.astype(np.float16),
            "maskm": maskm.astype(np.float16), "maskb": maskb,
        })

    trace = bool(int(os.environ.get("KERNEL_PROFILE", "0")))
    res = run_bass_kernel_spmd(nc, in_maps, core_ids=list(range(8)),
                               trace=trace,
                               trace_cores=list(range(8)) if trace else None)
    last_results = res

    out = np.empty((B, S, D), dtype=np.float32)
    for b in range(B):
        acc = (res.results[2 * b]["outT"].astype(np.float32)
               + res.results[2 * b + 1]["outT"].astype(np.float32))
        out[b] = acc.T
    # V-bias epilogue: softmax rows sum to 1, so the V bias contributes a
    # constant (b_v @ w_o.T) to every sequence position.
    out += (b_qkv[2 * D:] @ woT_full).reshape(1, 1, D)
    return out
